# revision 2
# baseline (speedup 1.0000x reference)
"""Chamfer distance loss kernel for 8 Trainium2 NeuronCores.

Problem: points1 [8, 4096, 3], points2 [8, 4096, 3] (f32).
  dist[b,n,m] = ||p1[b,n]||^2 + ||p2[b,m]||^2 - 2 p1.p2
  loss = (mean_n,b(min_m dist) + mean_m,b(min_n dist)) / 8     (scalar f32)

Sharding: data-parallel over batch B: core b handles batch b.

Per-core algorithm (flash-style, nothing materialized in HBM):
  Host lifts each point cloud to K=8 rows so that the *negated* distance
  matrix is one K=8 matmul:  -d[n,m] = sum_k la[k,n] * lb[k,m]
     la[:,n] = [sq1[n], 1, x1, y1, z1, 0,0,0]
     lb[:,m] = [-1, -sq2[m], 2*x2, 2*y2, 2*z2, 0,0,0]
  (negated so every reduction is a MAX)
  Device loop over 32 row-strips of 128 points1:
     PE:  8 matmuls (N=512, fp32, 4-way packed via tile_position along the
          K dim of the PE array) -> PSUM strip [128, 4096] f32 (2 halves)
     ACT: cast PSUM f32 -> SBUF fp16 strip (2 ops, one per half)
     DVE: colacc = max(colacc, strip) elementwise (TT, fp16 2x mode)
          rowmax[strip] via ONE tensor_scalar(op0=max, op1=max,
          accum_out=[128,1]) in fp16 4x mode (in-place, replaces the old
          fold-max tree: 1024 cycles vs ~1984)
  Tail: colacc partition-max via 32 PE transposes into one PSUM f16 tile
        [128, 32, 128], then 4 quarter tensor_reduce(max) -> [128, 8] each
        (quarter-split so transposes overlap the last strip), fused sum,
        gpsimd partition reduce, one f32 scalar ( -(rowsum+colsum) ).
Host: loss = -sum(partials) / (B*B*N).
"""

import sys
import numpy as np

for _p in ("/opt/trn_rl_repo", "/root/.axon_site/_ro/trn_rl_repo"):
    if _p not in sys.path:
        sys.path.insert(0, _p)

B = 8
N = 4096
D = 3
K = 8
P = 128
NSTRIP = N // P          # 32
MM_FREE = 512            # fp32 matmul moving-operand max
MHALF = 2048             # half strip (4 PSUM banks)

_NC_CACHE = {}


def _build_nc(repeat=1):
    """Build the per-core bass program.

    repeat: wrap the whole compute body in an on-device For_i loop (used
        only for timing: slope over `repeat` isolates device time from the
        ~5ms axon launch overhead).
    """
    import contextlib

    import concourse.bacc as bacc
    import concourse.tile as tile
    from concourse import bass_isa, mybir

    F16 = mybir.dt.float16
    F32 = mybir.dt.float32
    MAX = mybir.AluOpType.max
    ADD = mybir.AluOpType.add

    nc = bacc.Bacc(
        "TRN2", target_bir_lowering=False, debug=False, num_devices=B
    )
    la = nc.declare_dram_parameter("la", [K, N], F32, isOutput=False)
    lb = nc.declare_dram_parameter("lb", [K, N], F32, isOutput=False)
    ident = nc.declare_dram_parameter("ident", [P, P], F16, isOutput=False)
    out = nc.declare_dram_parameter("partial", [1, 1], F32, isOutput=True)

    with tile.TileContext(nc) as tc:
        with (
            tc.tile_pool(name="consts", bufs=1) as consts,
            tc.tile_pool(name="strips", bufs=3) as strips,
            tc.tile_pool(name="accs", bufs=1) as accs,
            tc.tile_pool(name="psum", bufs=2, space="PSUM") as psum,
        ):
            # 4 copies of the lifted tensors at partition offsets
            # 0/32/64/96 so 4 matmuls can run in distinct PE row-groups.
            la_sb = consts.tile([3 * 32 + K, N], F32)
            lb_sb = consts.tile([3 * 32 + K, N], F32)
            # parallel input load: la on the SP HWDGE queue, lb on the
            # Activation HWDGE queue (the only two HWDGE engines)
            for q in range(4):
                nc.sync.dma_start(out=la_sb[32 * q : 32 * q + K, :], in_=la[:])
                nc.scalar.dma_start(out=lb_sb[32 * q : 32 * q + K, :], in_=lb[:])
            idt = consts.tile([P, P], F16)
            nc.gpsimd.dma_start(out=idt[:], in_=ident[:])

            loop_ctx = (
                tc.For_i(0, repeat, 1) if repeat != 1 else contextlib.nullcontext()
            )
            with loop_ctx:
                colacc = accs.tile([P, N], F16)
                rowred = accs.tile([P, NSTRIP], F32)
                colred = accs.tile([P, NSTRIP], F32)

                def emit_mms(i, h, ph):
                    for j in range(MHALF // MM_FREE):
                        m0 = j * MM_FREE
                        nc.tensor.matmul(
                            ph[:, m0 : m0 + MM_FREE],
                            lhsT=la_sb[32 * j : 32 * j + K, i * P : (i + 1) * P],
                            rhs=lb_sb[
                                32 * j : 32 * j + K,
                                h * MHALF + m0 : h * MHALF + m0 + MM_FREE,
                            ],
                            start=True,
                            stop=True,
                            tile_position=(32 * j, 0),
                        )

                for i in range(NSTRIP):
                    strip = strips.tile([P, N], F16, tag="strip")
                    for h in range(2):
                        ph = psum.tile([P, MHALF], F32, tag="ph")
                        emit_mms(i, h, ph)
                        # cast f32 PSUM -> f16 SBUF (ScalarE/ACT)
                        nc.scalar.copy(
                            strip[:, h * MHALF : (h + 1) * MHALF], ph[:]
                        )
                    if i == 0:
                        # first strip initializes colacc (tensor_copy runs
                        # in the 4x DVE mode; replaces a memset + max)
                        nc.vector.tensor_copy(colacc[:], strip[:])
                    elif i == NSTRIP - 1:
                        # final colmax split by m-quarters so the tail's PE
                        # transposes can start per-quarter
                        for q in range(4):
                            qs = q * (N // 4)
                            qe = qs + N // 4
                            nc.vector.tensor_tensor(
                                colacc[:, qs:qe],
                                colacc[:, qs:qe],
                                strip[:, qs:qe],
                                op=MAX,
                            )
                    else:
                        nc.vector.tensor_tensor(
                            colacc[:], colacc[:], strip[:], op=MAX
                        )
                    # row-max of the whole strip in one DVE 4x op: in-place
                    # tensor_scalar, accum_out reduces along free with op1
                    nc.vector.tensor_scalar(
                        out=strip[:],
                        in0=strip[:],
                        scalar1=-1.0e30,
                        scalar2=None,
                        op0=MAX,
                        op1=MAX,
                        accum_out=rowred[:, i : i + 1],
                    )

                # ---- tail ----
                # colacc partition-max: 32 PE transposes into one PSUM f16
                # tile, then per-quarter tensor_reduce(max) over [P, 8, 128]
                tailp = psum.tile([P, NSTRIP, P], F16, tag="ph")
                for q in range(4):
                    for t in range(8):
                        k = 8 * q + t
                        nc.tensor.transpose(
                            tailp[:, k, :], colacc[:, k * P : (k + 1) * P], idt[:]
                        )
                    nc.vector.tensor_reduce(
                        out=colred[:, 8 * q : 8 * q + 8],
                        in_=tailp[:, 8 * q : 8 * q + 8, :],
                        axis=mybir.AxisListType.X,
                        op=MAX,
                    )

                summ = accs.tile([P, 2], F32)
                nc.vector.tensor_reduce(
                    out=summ[:, 0:1], in_=rowred[:], axis=mybir.AxisListType.X,
                    op=ADD,
                )
                nc.vector.tensor_reduce(
                    out=summ[:, 1:2], in_=colred[:], axis=mybir.AxisListType.X,
                    op=ADD,
                )
                tot = accs.tile([P, 1], F32)
                nc.vector.tensor_tensor(tot[:], summ[:, 0:1], summ[:, 1:2], op=ADD)
                tot_red = accs.tile([P, 1], F32)
                nc.gpsimd.partition_all_reduce(
                    tot_red[:], tot[:], P, bass_isa.ReduceOp.add
                )
                nc.sync.dma_start(out=out[:], in_=tot_red[0:1, :])

    nc.compile()
    return nc


def get_nc(repeat=1, **_ignored):
    key = repeat
    if key not in _NC_CACHE:
        _NC_CACHE[key] = _build_nc(repeat=repeat)
    return _NC_CACHE[key]


def _lift(points1, points2):
    """Host-side O(N) prep: lifted vectors so -dist = la^T @ lb."""
    p1 = np.asarray(points1, dtype=np.float32)
    p2 = np.asarray(points2, dtype=np.float32)
    sq1 = np.sum(p1 * p1, axis=-1)  # [B, N]
    sq2 = np.sum(p2 * p2, axis=-1)  # [B, N]
    la = np.zeros((B, K, N), dtype=np.float32)
    lb = np.zeros((B, K, N), dtype=np.float32)
    la[:, 0, :] = sq1
    la[:, 1, :] = 1.0
    la[:, 2:5, :] = np.transpose(p1, (0, 2, 1))
    lb[:, 0, :] = -1.0
    lb[:, 1, :] = -sq2
    lb[:, 2:5, :] = 2.0 * np.transpose(p2, (0, 2, 1))
    return la, lb


def _in_maps(points1, points2):
    la, lb = _lift(points1, points2)
    ident = np.eye(P, dtype=np.float16)
    return [
        {
            "la": np.ascontiguousarray(la[b]),
            "lb": np.ascontiguousarray(lb[b]),
            "ident": ident,
        }
        for b in range(B)
    ]


def kernel(points1, points2):
    from concourse.bass_utils import run_bass_kernel_spmd

    in_maps = _in_maps(points1, points2)
    nc = get_nc()
    res = run_bass_kernel_spmd(nc, in_maps, list(range(B))).results
    tot = -sum(float(res[b]["partial"][0, 0]) for b in range(B))
    loss = tot / (B * B * N)
    return np.float32(loss)
